# revision 36
# baseline (speedup 1.0000x reference)
"""Trainium2 Bass kernel for nn_DynamicConv (per-pixel dynamic 5x5 conv, 8 heads).

Reference computation (per batch image b):
    f[i, j, :]  = sum_c x[b, c, i, j] * filt_w[c, :]          # (56,56,200)
    out[c, i, j] = sum_{kh,kw} xpad[c, i+kh, j+kw] * f[i, j, kh, kw, c//24]

Sharding: data-parallel over batch, but each core takes 2 images x one
28-column half of the width so that 112 of 128 SBUF partitions carry
(row, image) pairs: partition q = 2*row + img.  Compute-engine APs must
start at partition 0, so the five kh row shifts are materialized as five
separately-laid-out DRAM loads x_d0..x_d4
(x_dk[q, c, jp] = xpad[img, c, i+k, jp]); the kw shifts are free-dim
offsets.

Pipeline (VERSION=9, bf16 datapath):
 - All inputs host-converted to bf16.  x_gen/fw DMAs go out alone first
   (fgen starts ~6us in); the ten x_d half-loads are triggered from the
   otherwise-idle GPSIMD queue behind tiny gate-reads so they never
   steal bandwidth from x_gen: the five channel-lo halves (all conv
   half 0 needs) right after x_gen lands, the hi halves after those.
 - PE: filter generation — per output column j, a (96ch x 112px) slice
   of a channel-major copy of x is the stationary operand against
   filt_w (96 x 200), accumulating j-pairs x 2 channel-chunks in PSUM;
   ACT evacuates a j-pair per copy into bf16 f_sb in (kl, head, j)
   order.
 - DVE: the 25 tap products per channel-half, one (112, 4head, 24, 28)
   broadcast-multiply each, all-bf16 so the DVE 2x mode engages (no
   GPSIMD column split: concurrent GPSIMD multiplies contend for SBUF
   ports and halve BOTH engines).
 - PE: sums the 25 bf16 products per half via bf16 identity matmuls
   accumulating fp32 in PSUM (6 x 448-column chunks); ACT evacuates
   each chunk to fp32 acc and SP DMAs it out immediately.
"""

import numpy as np

import concourse.bass as bass
import concourse.bacc as bacc
import concourse.mybir as mybir
import concourse.tile as tile
from concourse.bass_utils import run_bass_kernel_spmd

B, C, H, W = 8, 192, 56, 56
K, HEADS = 5, 8
CG = C // HEADS            # 24 channels per head
FCOLS = K * K * HEADS      # 200 filter-gen outputs per pixel
WH = 28                    # columns per core (half width)
JP = WH + 4                # padded columns held in SBUF
P_O = 2 * H                # 112 partitions carrying (row, img) pairs
N_CORES = 8

F32 = mybir.dt.float32
BF16 = mybir.dt.bfloat16

HHEADS = HEADS // 2        # heads per channel-half
NCH = 6                    # PSUM chunks per half
CHF = 96 * WH // NCH       # 448 fp32 per chunk = 16 channels x 28 cols
JB = 2                     # fgen j-columns batched per PSUM tile / evac


def build_nc():
    nc = bacc.Bacc(None)

    # packed per-partition lines: [fw (2x200) | xg j-half A | xg j-half B]
    # so one descriptor per partition carries filter weights + gen pixels
    XGH = (WH // 2) * 2 * P_O          # elems per xg j-half chunk
    XGF = 2 * FCOLS + 2 * XGH
    xgf_in = nc.dram_tensor("xgf", [96, XGF], BF16, kind="ExternalInput")
    idb_in = nc.dram_tensor("identb", [P_O, P_O], BF16, kind="ExternalInput")
    xd_in = [
        nc.dram_tensor(f"x_d{k}", [P_O, C, JP], BF16, kind="ExternalInput")
        for k in range(K)
    ]
    out_d = nc.dram_tensor("out_sbl", [P_O, C, WH], BF16, kind="ExternalOutput")

    with tile.TileContext(nc) as tc:
        with (
            tc.tile_pool(name="big", bufs=1) as big,
            tc.tile_pool(name="sh", bufs=4) as sh,
        ):
            xgf = big.tile([96, XGF], BF16)
            fw_sb = xgf[:, 0 : 2 * FCOLS].rearrange("c (k f) -> c k f", k=2)
            xg = xgf[:, 2 * FCOLS :].rearrange(
                "c (j k p) -> c j k p", k=2, p=P_O
            )
            identb = big.tile([P_O, P_O], BF16)
            gate = big.tile([1, 8], BF16)
            xd = [
                big.tile([P_O, C, JP], BF16, tag=f"xd{k}", name=f"xd{k}")
                for k in range(K)
            ]
            # (kl, h, j) order: conv in1 gets a contiguous innermost AP dim
            f_sb = big.tile([P_O, K * K, HEADS, WH], BF16)
            acc = big.tile([P_O, C, WH], BF16)
            # pad keeps the sh pool base where the fp32 acc had it
            pad = big.tile([P_O, C * WH], BF16)

            # fw is packed into xg chunk A's partition lines (one descriptor
            # per partition carries both); chunk B streams while fgen starts
            # on A.  identb follows (needed only by the first identsum).
            CA = 2 * FCOLS + XGH
            nc.gpsimd.memset(pad[0:1, 0:8], 0)
            nc.sync.dma_start(xgf[:, :CA], xgf_in[:, :CA])
            nc.sync.dma_start(xgf[:, CA:], xgf_in[:, CA:])
            # identb after BOTH xgf chunks: its 112 tiny descriptors would
            # otherwise sit between them and delay chunk B ~2.4us
            nc.sync.dma_start(identb[:], idb_in[:])
            # Gate the big x_d loads behind x_gen via the idle GPSIMD queue,
            # chained so xd0's lo half lands first (conv consumes xd[k] lo
            # halves in k order); hi halves follow after all lo halves.
            nc.gpsimd.tensor_copy(gate[:], xgf[0:1, XGF - 8 : XGF])
            for k in range(K):
                nc.gpsimd.dma_start(xd[k][:, 0:96, :], xd_in[k][:, 0:96, :])
                if k < K - 1:
                    nc.gpsimd.tensor_copy(gate[:], xd[k][0:1, 95, 0:8])
            nc.gpsimd.tensor_copy(gate[:], xd[K - 1][0:1, 95, 0:8])
            for k in range(K):
                nc.gpsimd.dma_start(xd[k][:, 96:192, :], xd_in[k][:, 96:192, :])

            # ---- PE warm-up + filter generation, in their own PSUM pool
            # scope so fgen can triple-buffer without starving the conv's
            # six accumulation banks.
            with tc.tile_pool(name="ps_f", bufs=3, space="PSUM") as ps_f:
                # Input-independent dummy matmuls keep the PE continuously
                # busy until x_gen lands, so its p-state ramps to full clock
                # and fgen runs at ~86ns instead of 167ns per matmul.
                warm = big.tile([P_O, P_O], BF16)
                nc.vector.memset(warm[:], 0)
                wps = ps_f.tile([P_O, P_O], F32, tag="fps")
                NW = 64
                for w in range(NW):
                    nc.tensor.matmul(
                        wps[:], warm[:], warm[:],
                        start=(w == 0), stop=(w == NW - 1),
                    )

                # f[q, kk, h, j] = sum_c x[c, q, j] * fw[c, kk*8+h]
                for jb in range(WH // JB):
                    if jb == (WH // 2) // JB:
                        # filler matmuls bridge any wait for xgf chunk B so
                        # the PE stays hot for the second fgen half
                        wps2 = ps_f.tile([P_O, P_O], F32, tag="fps")
                        for w in range(8):
                            nc.tensor.matmul(
                                wps2[:], warm[:], warm[:],
                                start=(w == 0), stop=(w == 7),
                            )
                    fps = ps_f.tile([P_O, JB, K * K, HEADS], F32, tag="fps")
                    for jj in range(JB):
                        j = jb * JB + jj
                        for ck in range(2):
                            nc.tensor.matmul(
                                fps[:, jj, :, :],
                                xg[:, j, ck, :],   # (96 ch, 112 px) stationary
                                fw_sb[:, ck, :],   # (96 ch, 200)
                                start=(ck == 0),
                                stop=(ck == 1),
                            )
                    # alternate ACT/DVE so the evac never stalls the PE's
                    # p-state ramp (DVE is idle until conv starts anyway)
                    evac = (
                        nc.scalar.copy if jb % 2 == 0 else nc.vector.tensor_copy
                    )
                    evac(
                        f_sb[:, :, :, jb * JB : (jb + 1) * JB],
                        fps[:].rearrange("q j kl h -> q kl h j"),
                    )

            # ---- conv: DVE computes the 25 bf16 products per channel-half;
            # the PE sums them with bf16 identity matmuls accumulating fp32
            # in PSUM; ACT evacuates per chunk, SP DMAs each chunk out.
            ps_a_cm = tc.tile_pool(name="ps_a", bufs=NCH, space="PSUM")
            ps_a = ps_a_cm.__enter__()
            for hh in range(2):
                c0 = hh * 96
                accps = [
                    ps_a.tile([P_O, CHF], F32, tag="accps", name=f"accps{hh}_{b}")
                    for b in range(NCH)
                ]
                for kl in range(K * K):
                    kh, kw = divmod(kl, K)
                    xin = xd[kh][:, c0 : c0 + 96, kw : kw + WH]
                    xin4 = xin.rearrange("p (h g) j -> p h g j", h=HHEADS)
                    fbc = (
                        f_sb[:, kl, hh * HHEADS : (hh + 1) * HHEADS, :]
                        .unsqueeze(2)
                        .broadcast_to([P_O, HHEADS, CG, WH])
                    )
                    prod = sh.tile(
                        [P_O, 96, WH], BF16, tag="xgprod", name=f"prod{hh}_{kl}",
                    )
                    p4 = prod[:].rearrange("p (h g) j -> p h g j", h=HHEADS)
                    nc.vector.tensor_mul(p4, xin4, fbc)
                    pflat = prod[:].rearrange("p c j -> p (c j)")
                    for b in range(NCH):
                        nc.tensor.matmul(
                            accps[b][:],
                            identb[:],
                            pflat[:, b * CHF : (b + 1) * CHF],
                            start=(kl == 0),
                            stop=(kl == K * K - 1),
                        )
                for b in range(NCH):
                    cl = c0 + b * 16
                    # last half: DVE is idle, split the evac to trim the tail
                    if hh == 1 and b % 2 == 1:
                        nc.vector.tensor_copy(
                            acc[:, cl : cl + 16, :],
                            accps[b][:].rearrange("p (c j) -> p c j", j=WH),
                        )
                    else:
                        nc.scalar.copy(
                            acc[:, cl : cl + 16, :],
                            accps[b][:].rearrange("p (c j) -> p c j", j=WH),
                        )
                    nc.sync.dma_start(
                        out_d[:, cl : cl + 16, :], acc[:, cl : cl + 16, :]
                    )
            ps_a_cm.__exit__(None, None, None)

    return nc


def shard_inputs(x, filt_w):
    """Split full inputs into the 8 per-core input maps."""
    import ml_dtypes

    bf16 = ml_dtypes.bfloat16
    x = np.ascontiguousarray(np.asarray(x, dtype=np.float32))
    fw = np.ascontiguousarray(np.asarray(filt_w, dtype=np.float32))
    fw_pk = np.ascontiguousarray(
        fw.reshape(2, 96, FCOLS).transpose(1, 0, 2)
    ).astype(bf16)
    identb = np.eye(P_O).astype(bf16)

    in_maps = []
    for core in range(N_CORES):
        pair, jh = divmod(core, 2)
        xs = x[2 * pair : 2 * pair + 2]           # (2, C, 56, 56)
        xpad = np.zeros((2, C, H + 4, JP), np.float32)
        lo = jh * WH - 2                           # global col of jp=0
        s_lo, s_hi = max(lo, 0), min(lo + JP, W)
        xpad[:, :, 2 : 2 + H, s_lo - lo : s_lo - lo + (s_hi - s_lo)] = xs[
            :, :, :, s_lo:s_hi
        ]
        m = {"identb": identb}
        for k in range(K):
            # x_dk[2*i+img, c, jp] = xpad[img, c, i+k, jp]
            m[f"x_d{k}"] = np.ascontiguousarray(
                xpad[:, :, k : k + H, :].transpose(2, 0, 1, 3).reshape(P_O, C, JP)
            ).astype(bf16)
        # channel-major copy for filter-gen: x_gen[c96, j, ck, 2*i+img],
        # packed per partition as [fw (2x200) | xg j-half A | xg j-half B]
        xs_half = xs[:, :, :, jh * WH : (jh + 1) * WH]  # (2, C, 56, 28)
        xg = xs_half.transpose(1, 3, 2, 0).reshape(C, WH, P_O)
        xg = xg.reshape(2, 96, WH, P_O).transpose(1, 2, 0, 3)  # (96, j, ck, q)
        m["xgf"] = np.ascontiguousarray(
            np.concatenate(
                [fw_pk.reshape(96, -1), xg.reshape(96, -1)], axis=1
            )
        ).astype(bf16)
        in_maps.append(m)
    return in_maps


def unshard_output(results):
    """Reassemble the 8 per-core outputs into the full (B, C, H, W) tensor."""
    out = np.empty((B, C, H, W), np.float32)
    for core in range(N_CORES):
        pair, jh = divmod(core, 2)
        arr = np.asarray(results[core]["out_sbl"]).astype(np.float32)
        arr = arr.reshape(H, 2, C, WH)
        # arr[i, img, c, j] = out[2*pair+img, c, i, jh*28+j]
        out[2 * pair : 2 * pair + 2, :, :, jh * WH : (jh + 1) * WH] = arr.transpose(
            1, 2, 0, 3
        )
    return out


_NC_CACHE = None


def _get_nc():
    global _NC_CACHE
    if _NC_CACHE is None:
        _NC_CACHE = build_nc()
        if not _NC_CACHE.is_finalized():
            _NC_CACHE.finalize()
    return _NC_CACHE


def run(inputs, trace=False, **kwargs):
    """Run on the 8 NeuronCores; returns BassKernelResults."""
    in_maps = shard_inputs(inputs["x"], inputs["filt_w"])
    nc = _get_nc()
    return run_bass_kernel_spmd(
        nc, in_maps, core_ids=list(range(N_CORES)), trace=trace, **kwargs
    )


def kernel(x, filt_w):
    res = run({"x": x, "filt_w": filt_w})
    return unshard_output(res.results)


# revision 37
# speedup vs baseline: 1.1718x; 1.1718x over previous
"""Trainium2 Bass kernel for nn_DynamicConv (per-pixel dynamic 5x5 conv, 8 heads).

Reference computation (per batch image b):
    f[i, j, :]  = sum_c x[b, c, i, j] * filt_w[c, :]          # (56,56,200)
    out[c, i, j] = sum_{kh,kw} xpad[c, i+kh, j+kw] * f[i, j, kh, kw, c//24]

Sharding: data-parallel over batch, but each core takes 2 images x one
28-column half of the width so that 112 of 128 SBUF partitions carry
(row, image) pairs: partition q = 2*row + img.  Compute-engine APs must
start at partition 0, so the five kh row shifts are materialized as five
separately-laid-out DRAM loads x_d0..x_d4
(x_dk[q, c, jp] = xpad[img, c, i+k, jp]); the kw shifts are free-dim
offsets.

Pipeline (VERSION=9, bf16 datapath):
 - All inputs host-converted to bf16.  x_gen/fw DMAs go out alone first
   (fgen starts ~6us in); the ten x_d half-loads are triggered from the
   otherwise-idle GPSIMD queue behind tiny gate-reads so they never
   steal bandwidth from x_gen: the five channel-lo halves (all conv
   half 0 needs) right after x_gen lands, the hi halves after those.
 - PE: filter generation — per output column j, a (96ch x 112px) slice
   of a channel-major copy of x is the stationary operand against
   filt_w (96 x 200), accumulating j-pairs x 2 channel-chunks in PSUM;
   ACT evacuates a j-pair per copy into bf16 f_sb in (kl, head, j)
   order.
 - DVE: the 25 tap products per channel-half, one (112, 4head, 24, 28)
   broadcast-multiply each, all-bf16 so the DVE 2x mode engages (no
   GPSIMD column split: concurrent GPSIMD multiplies contend for SBUF
   ports and halve BOTH engines).
 - PE: sums the 25 bf16 products per half via bf16 identity matmuls
   accumulating fp32 in PSUM (6 x 448-column chunks); ACT evacuates
   each chunk to fp32 acc and SP DMAs it out immediately.
"""

import numpy as np

import concourse.bass as bass
import concourse.bacc as bacc
import concourse.mybir as mybir
import concourse.tile as tile
from concourse.bass_utils import run_bass_kernel_spmd

B, C, H, W = 8, 192, 56, 56
K, HEADS = 5, 8
CG = C // HEADS            # 24 channels per head
FCOLS = K * K * HEADS      # 200 filter-gen outputs per pixel
WH = 28                    # columns per core (half width)
JP = WH + 4                # padded columns held in SBUF
P_O = 2 * H                # 112 partitions carrying (row, img) pairs
N_CORES = 8

F32 = mybir.dt.float32
BF16 = mybir.dt.bfloat16

HHEADS = HEADS // 2        # heads per channel-half
NCH = 6                    # PSUM chunks per half
CHF = 96 * WH // NCH       # 448 fp32 per chunk = 16 channels x 28 cols
JB = 2                     # fgen j-columns batched per PSUM tile / evac


def build_nc():
    nc = bacc.Bacc(None)

    # packed per-partition lines: [fw (2x200) | xg j-half A | xg j-half B]
    # so one descriptor per partition carries filter weights + gen pixels
    XGH = (WH // 2) * 2 * P_O          # elems per xg j-half chunk
    XGF = 2 * FCOLS + 2 * XGH
    xgf_in = nc.dram_tensor("xgf", [96, XGF], BF16, kind="ExternalInput")
    xd_in = [
        nc.dram_tensor(f"x_d{k}", [P_O, C, JP], BF16, kind="ExternalInput")
        for k in range(K)
    ]
    out_d = nc.dram_tensor("out_sbl", [P_O, C, WH], BF16, kind="ExternalOutput")

    with tile.TileContext(nc) as tc:
        with (
            tc.tile_pool(name="big", bufs=1) as big,
            tc.tile_pool(name="sh", bufs=4) as sh,
        ):
            xgf = big.tile([96, XGF], BF16)
            fw_sb = xgf[:, 0 : 2 * FCOLS].rearrange("c (k f) -> c k f", k=2)
            xg = xgf[:, 2 * FCOLS :].rearrange(
                "c (j k p) -> c j k p", k=2, p=P_O
            )
            identb = big.tile([P_O, P_O], BF16)
            gate = big.tile([1, 8], BF16)
            xd = [
                big.tile([P_O, C, JP], BF16, tag=f"xd{k}", name=f"xd{k}")
                for k in range(K)
            ]
            # (kl, h, j) order: conv in1 gets a contiguous innermost AP dim
            f_sb = big.tile([P_O, K * K, HEADS, WH], BF16)
            acc = big.tile([P_O, C, WH], BF16)
            # pad keeps the sh pool base where the fp32 acc had it
            pad = big.tile([P_O, C * WH], BF16)

            # fw is packed into xg chunk A's partition lines (one descriptor
            # per partition carries both); chunk B streams while fgen starts
            # on A.
            CA = 2 * FCOLS + XGH
            nc.gpsimd.memset(pad[0:1, 0:8], 0)
            nc.sync.dma_start(xgf[:, :CA], xgf_in[:, :CA])
            nc.sync.dma_start(xgf[:, CA:], xgf_in[:, CA:])
            # identb is built on-chip (a DMA's 112 tiny descriptors would
            # interleave with and delay xgf chunk B on the queues)
            ones = big.tile([P_O, P_O], BF16)
            nc.gpsimd.memset(ones[:], 1.0)
            nc.gpsimd.affine_select(
                identb[:], ones[:], [[-1, P_O]],
                mybir.AluOpType.is_equal, 0.0,
                base=0, channel_multiplier=1,
            )
            # Gate the big x_d loads behind x_gen via the idle GPSIMD queue,
            # chained so xd0's lo half lands first (conv consumes xd[k] lo
            # halves in k order); hi halves follow after all lo halves.
            nc.gpsimd.tensor_copy(gate[:], xgf[0:1, XGF - 8 : XGF])
            for k in range(K):
                nc.gpsimd.dma_start(xd[k][:, 0:96, :], xd_in[k][:, 0:96, :])
                if k < K - 1:
                    nc.gpsimd.tensor_copy(gate[:], xd[k][0:1, 95, 0:8])
            nc.gpsimd.tensor_copy(gate[:], xd[K - 1][0:1, 95, 0:8])
            for k in range(K):
                nc.gpsimd.dma_start(xd[k][:, 96:192, :], xd_in[k][:, 96:192, :])

            # ---- PE warm-up + filter generation, in their own PSUM pool
            # scope so fgen can triple-buffer without starving the conv's
            # six accumulation banks.
            with tc.tile_pool(name="ps_f", bufs=3, space="PSUM") as ps_f:
                # Input-independent dummy matmuls keep the PE continuously
                # busy until x_gen lands, so its p-state ramps to full clock
                # and fgen runs at ~86ns instead of 167ns per matmul.
                warm = big.tile([P_O, P_O], BF16)
                nc.vector.memset(warm[:], 0)
                wps = ps_f.tile([P_O, P_O], F32, tag="fps")
                NW = 64
                for w in range(NW):
                    nc.tensor.matmul(
                        wps[:], warm[:], warm[:],
                        start=(w == 0), stop=(w == NW - 1),
                    )

                # f[q, kk, h, j] = sum_c x[c, q, j] * fw[c, kk*8+h]
                for jb in range(WH // JB):
                    if jb == (WH // 2) // JB:
                        # filler matmuls bridge any wait for xgf chunk B so
                        # the PE stays hot for the second fgen half
                        wps2 = ps_f.tile([P_O, P_O], F32, tag="fps")
                        for w in range(8):
                            nc.tensor.matmul(
                                wps2[:], warm[:], warm[:],
                                start=(w == 0), stop=(w == 7),
                            )
                    fps = ps_f.tile([P_O, JB, K * K, HEADS], F32, tag="fps")
                    for jj in range(JB):
                        j = jb * JB + jj
                        for ck in range(2):
                            nc.tensor.matmul(
                                fps[:, jj, :, :],
                                xg[:, j, ck, :],   # (96 ch, 112 px) stationary
                                fw_sb[:, ck, :],   # (96 ch, 200)
                                start=(ck == 0),
                                stop=(ck == 1),
                            )
                    # alternate ACT/DVE so the evac never stalls the PE's
                    # p-state ramp (DVE is idle until conv starts anyway)
                    evac = (
                        nc.scalar.copy if jb % 2 == 0 else nc.vector.tensor_copy
                    )
                    evac(
                        f_sb[:, :, :, jb * JB : (jb + 1) * JB],
                        fps[:].rearrange("q j kl h -> q kl h j"),
                    )

            # ---- conv: DVE computes the 25 bf16 products per channel-half;
            # the PE sums them with bf16 identity matmuls accumulating fp32
            # in PSUM; ACT evacuates per chunk, SP DMAs each chunk out.
            ps_a_cm = tc.tile_pool(name="ps_a", bufs=NCH, space="PSUM")
            ps_a = ps_a_cm.__enter__()
            for hh in range(2):
                c0 = hh * 96
                accps = [
                    ps_a.tile([P_O, CHF], F32, tag="accps", name=f"accps{hh}_{b}")
                    for b in range(NCH)
                ]
                for kl in range(K * K):
                    kh, kw = divmod(kl, K)
                    xin = xd[kh][:, c0 : c0 + 96, kw : kw + WH]
                    xin4 = xin.rearrange("p (h g) j -> p h g j", h=HHEADS)
                    fbc = (
                        f_sb[:, kl, hh * HHEADS : (hh + 1) * HHEADS, :]
                        .unsqueeze(2)
                        .broadcast_to([P_O, HHEADS, CG, WH])
                    )
                    prod = sh.tile(
                        [P_O, 96, WH], BF16, tag="xgprod", name=f"prod{hh}_{kl}",
                    )
                    p4 = prod[:].rearrange("p (h g) j -> p h g j", h=HHEADS)
                    nc.vector.tensor_mul(p4, xin4, fbc)
                    pflat = prod[:].rearrange("p c j -> p (c j)")
                    for b in range(NCH):
                        nc.tensor.matmul(
                            accps[b][:],
                            identb[:],
                            pflat[:, b * CHF : (b + 1) * CHF],
                            start=(kl == 0),
                            stop=(kl == K * K - 1),
                        )
                for b in range(NCH):
                    cl = c0 + b * 16
                    # last half: DVE is idle, split the evac to trim the tail
                    if hh == 1 and b % 2 == 1:
                        nc.vector.tensor_copy(
                            acc[:, cl : cl + 16, :],
                            accps[b][:].rearrange("p (c j) -> p c j", j=WH),
                        )
                    else:
                        nc.scalar.copy(
                            acc[:, cl : cl + 16, :],
                            accps[b][:].rearrange("p (c j) -> p c j", j=WH),
                        )
                    nc.sync.dma_start(
                        out_d[:, cl : cl + 16, :], acc[:, cl : cl + 16, :]
                    )
            ps_a_cm.__exit__(None, None, None)

    return nc


def shard_inputs(x, filt_w):
    """Split full inputs into the 8 per-core input maps."""
    import ml_dtypes

    bf16 = ml_dtypes.bfloat16
    x = np.ascontiguousarray(np.asarray(x, dtype=np.float32))
    fw = np.ascontiguousarray(np.asarray(filt_w, dtype=np.float32))
    fw_pk = np.ascontiguousarray(
        fw.reshape(2, 96, FCOLS).transpose(1, 0, 2)
    ).astype(bf16)

    in_maps = []
    for core in range(N_CORES):
        pair, jh = divmod(core, 2)
        xs = x[2 * pair : 2 * pair + 2]           # (2, C, 56, 56)
        xpad = np.zeros((2, C, H + 4, JP), np.float32)
        lo = jh * WH - 2                           # global col of jp=0
        s_lo, s_hi = max(lo, 0), min(lo + JP, W)
        xpad[:, :, 2 : 2 + H, s_lo - lo : s_lo - lo + (s_hi - s_lo)] = xs[
            :, :, :, s_lo:s_hi
        ]
        m = {}
        for k in range(K):
            # x_dk[2*i+img, c, jp] = xpad[img, c, i+k, jp]
            m[f"x_d{k}"] = np.ascontiguousarray(
                xpad[:, :, k : k + H, :].transpose(2, 0, 1, 3).reshape(P_O, C, JP)
            ).astype(bf16)
        # channel-major copy for filter-gen: x_gen[c96, j, ck, 2*i+img],
        # packed per partition as [fw (2x200) | xg j-half A | xg j-half B]
        xs_half = xs[:, :, :, jh * WH : (jh + 1) * WH]  # (2, C, 56, 28)
        xg = xs_half.transpose(1, 3, 2, 0).reshape(C, WH, P_O)
        xg = xg.reshape(2, 96, WH, P_O).transpose(1, 2, 0, 3)  # (96, j, ck, q)
        m["xgf"] = np.ascontiguousarray(
            np.concatenate(
                [fw_pk.reshape(96, -1), xg.reshape(96, -1)], axis=1
            )
        ).astype(bf16)
        in_maps.append(m)
    return in_maps


def unshard_output(results):
    """Reassemble the 8 per-core outputs into the full (B, C, H, W) tensor."""
    out = np.empty((B, C, H, W), np.float32)
    for core in range(N_CORES):
        pair, jh = divmod(core, 2)
        arr = np.asarray(results[core]["out_sbl"]).astype(np.float32)
        arr = arr.reshape(H, 2, C, WH)
        # arr[i, img, c, j] = out[2*pair+img, c, i, jh*28+j]
        out[2 * pair : 2 * pair + 2, :, :, jh * WH : (jh + 1) * WH] = arr.transpose(
            1, 2, 0, 3
        )
    return out


_NC_CACHE = None


def _get_nc():
    global _NC_CACHE
    if _NC_CACHE is None:
        _NC_CACHE = build_nc()
        if not _NC_CACHE.is_finalized():
            _NC_CACHE.finalize()
    return _NC_CACHE


def run(inputs, trace=False, **kwargs):
    """Run on the 8 NeuronCores; returns BassKernelResults."""
    in_maps = shard_inputs(inputs["x"], inputs["filt_w"])
    nc = _get_nc()
    return run_bass_kernel_spmd(
        nc, in_maps, core_ids=list(range(N_CORES)), trace=trace, **kwargs
    )


def kernel(x, filt_w):
    res = run({"x": x, "filt_w": filt_w})
    return unshard_output(res.results)
